# revision 23
# baseline (speedup 1.0000x reference)
"""
Trainium2 Bass kernel for DirectRankingModel:
    h = tanh(x @ W1.T + b1); s = (h @ W2.T + b2); e = exp(s)
    out = e / segment_sum(e, T)[T]    with 2 segments, N = 2,000,000 rows.

Strategy (8 NeuronCores, v3):
  - Host: rows are PARTITIONED BY GROUP across cores (stable sort by T):
    cores 0..k0-1 hold only group-0 rows, cores k0..7 only group-1 (k0=4
    for balanced random T).  The group boundary is padded to a core
    boundary (zero rows, corrected via a host-computed scalar), so every
    core is group-pure: no sel mask, segment sum == plain sum.
  - Host casts x to f16 and block-transposes into chunks of 8192 rows:
    xd[ch] = [128, 4096]; partitions 0-63 hold features of rows [0,4096),
    64-127 rows [4096,8192) -> row-split PE runs both halves concurrently.
  - 31 chunks (253952 rows) per core.
  - mm1: K=64 f16 matmuls, N=512, half-pairs on partition halves -> PSUM
    ph [128, 1536] (3 banks, 2 bufs).
  - tanh on ACT in 1536-wide instructions (PSUM->SBUF f16) -- ACT is the
    kernel's roofline (~0.93 ns/col/core measured).
  - mm2: 32-strip trick; block b -> psum partition b%128, strips cycle
    c=b%32 with col-group tile_position; exp per 128-block super-tile.
  - Sums: per-super-tile DVE reduce of e; ones-matmul partition reduce;
    per-core partial = onehot * S - padc2; 2-float HBM AllReduce.
  - normalize: out = E * (1/sum_of_my_group) -- one tensor_scalar per
    half, store halves on alternating HWDGE queues.
"""

import os
import sys

import numpy as np

for _p in ("/opt/trn_rl_repo", "/root/.axon_site/_ro/trn_rl_repo"):
    if os.path.isdir(_p) and _p not in sys.path:
        sys.path.insert(0, _p)

import concourse.bacc as bacc
import concourse.bass as bass
import concourse.tile as tile
from concourse import mybir
from concourse.bass_utils import run_bass_kernel_spmd

F16 = mybir.dt.float16
F32 = mybir.dt.float32
ALU = mybir.AluOpType
ACTF = mybir.ActivationFunctionType

N_CORES = 8
N_ROWS = 2_000_000
IN_DIM = 64
HID = 128

# Device-side geometry (per core).
Q = 256                     # rows per score-block
CH_ROWS = 8192              # rows per DMA chunk ([128, 4096] f16 = 1 MiB)
N_CH = 31                   # chunks per core
R_CORE = N_CH * CH_ROWS     # 253952 rows per core
N_PAD = N_CORES * R_CORE    # 2031616 rows total (padded)
MM_PER_CH = 16              # mm1 matmuls (512 rows each) per chunk
N_MM = N_CH * MM_PER_CH     # 496
PH_MMS = 3                  # mm1 outputs per PSUM tile -> ACT N=1536
PH_COLS = 512 * PH_MMS
BLK_PER_HT = PH_COLS // Q   # 6 score blocks per ht tile
N_BLK = R_CORE // Q         # 992
N_ST = (N_BLK + 127) // 128  # 8 super-tiles (last partial: 96 blocks)
COLS = N_ST * Q             # 2048 e/out columns per partition


def _ap(handle_ap, offset, dims):
    """Custom access pattern on a DRAM tensor: dims = [[step, count], ...]."""
    return bass.AP(tensor=handle_ap.tensor, offset=offset, ap=list(dims))


# DVE-assisted tanh: every ASSIST_EVERY-th full ht tile is computed on the
# vector engine with a minimax R(5,4) rational instead of ACT (the kernel's
# bottleneck): tanh(z) ~ z*(t^2 + A2 t + A0)/(t^2 + B2 t + B0), t = z^2,
# fit on |z| <= 4.8; f16 end-to-end max err ~2.7e-3.  The assisted tile's
# mm2 strip-groups are deferred ASSIST_DEFER tiles (whole 32-block groups,
# emitted atomically, so PSUM start/accumulate flags stay well-formed) to
# keep the in-order PE stream from ever waiting on the DVE chain.
ASSIST_EVERY = 11           # 0 disables
ASSIST_START = 8
ASSIST_STOP = 148
ASSIST_DEFER = 12
A2, A0 = -97.93266143005741, -1801.597436686564
B2, B0 = -680.3934051370769, -1811.5100012510918


def _is_assist(k):
    return (
        ASSIST_EVERY > 0
        and ASSIST_START <= k < ASSIST_STOP
        and (k - ASSIST_START) % ASSIST_EVERY == 0
    )


def build_nc(n_cores=N_CORES, use_coll=True):
    """Build the per-core Bass program (SPMD: same program, sliced inputs)."""
    from contextlib import ExitStack

    nc = bacc.Bacc(num_devices=n_cores)

    x_in = nc.declare_dram_parameter("x", [N_CH, 128, 4096], F16, isOutput=False)
    w1t_in = nc.declare_dram_parameter("w1t", [IN_DIM, HID], F16, isOutput=False)
    w2s_in = nc.declare_dram_parameter("w2s", [HID, 32 * 32], F16, isOutput=False)
    b1_in = nc.declare_dram_parameter("b1", [HID], F32, isOutput=False)
    b2_in = nc.declare_dram_parameter("b2", [1], F32, isOutput=False)
    oh_in = nc.declare_dram_parameter("oh", [2], F32, isOutput=False)
    pc2_in = nc.declare_dram_parameter("padc2", [2], F32, isOutput=False)
    out_t = nc.declare_dram_parameter("out", [128 * COLS], F32, isOutput=True)
    gs_t = nc.declare_dram_parameter("gsums", [2], F32, isOutput=True)

    cc_in = nc.dram_tensor("cc_in", [2], F32)
    cc_out = nc.dram_tensor("cc_out", [2], F32, addr_space="Shared")

    with ExitStack() as ctx:
        tc = ctx.enter_context(tile.TileContext(nc))
        singles = ctx.enter_context(tc.tile_pool(name="singles", bufs=1))
        xx_pool = ctx.enter_context(tc.tile_pool(name="xx", bufs=3))
        ht_pool = ctx.enter_context(
            tc.tile_pool(name="ht", bufs=(ASSIST_DEFER + 7) if ASSIST_EVERY else 6)
        )
        dv_pool = ctx.enter_context(tc.tile_pool(name="dv", bufs=1))
        gate_pool = ctx.enter_context(tc.tile_pool(name="gate", bufs=2))
        ph_pool = ctx.enter_context(tc.tile_pool(name="ph", bufs=2, space="PSUM"))
        ps_pool = ctx.enter_context(tc.tile_pool(name="ps", bufs=2, space="PSUM"))

        # ---- static setup ----------------------------------------------
        # First x chunk before anything else on the sync HWDGE queue; a
        # small leading slice so the first matmul can start earlier.
        xx_tiles = {}
        xx_tiles[0] = xx_pool.tile([128, 4096], F16, tag="xx", name="xx")
        w1t_sb = singles.tile([128, HID], F16)     # both halves hold W1T
        nc.sync.dma_start(
            out=w1t_sb[:],
            in_=_ap(w1t_in[:], 0, [[0, 2], [HID, IN_DIM], [1, HID]]),
        )
        # Fast-start slices: ph tile 0's mm1 reads (cols 0:1024), split
        # across both HWDGE queues.
        nc.sync.dma_start(
            out=xx_tiles[0][:, 0:512],
            in_=_ap(x_in[:], 0, [[4096, 128], [1, 512]]),
        )
        b1_sb = singles.tile([128, 1], F32)
        nc.sync.dma_start(out=b1_sb[:], in_=_ap(b1_in[:], 0, [[1, HID], [1, 1]]))
        b2_sb = singles.tile([128, 1], F32)
        nc.sync.dma_start(out=b2_sb[:], in_=_ap(b2_in[:], 0, [[0, 128], [1, 1]]))
        xx_tiles[1] = xx_pool.tile([128, 4096], F16, tag="xx", name="xx")
        nc.sync.dma_start(
            out=xx_tiles[1][:],
            in_=_ap(x_in[:], 128 * 4096, [[4096, 128], [1, 4096]]),
        )

        # Scalar HWDGE queue: rest of chunk 0, strips, small params.
        nc.scalar.dma_start(
            out=xx_tiles[0][:, 512:1024],
            in_=_ap(x_in[:], 512, [[4096, 128], [1, 512]]),
        )
        # 32 strip matrices [128, 32] fp16, strip c has W2 in column c.
        strips = singles.tile([128, 32, 32], F16)
        nc.scalar.dma_start(
            out=strips[:], in_=_ap(w2s_in[:], 0, [[32 * 32, HID], [1, 32 * 32]])
        )
        nc.scalar.dma_start(
            out=xx_tiles[0][:, 1024:4096],
            in_=_ap(x_in[:], 1024, [[4096, 128], [1, 3072]]),
        )
        oh1_sb = singles.tile([1, 2], F32)
        nc.scalar.dma_start(out=oh1_sb[:], in_=_ap(oh_in[:], 0, [[2, 1], [1, 2]]))
        pc2_sb = singles.tile([1, 2], F32)
        nc.scalar.dma_start(out=pc2_sb[:], in_=_ap(pc2_in[:], 0, [[2, 1], [1, 2]]))

        e_sb = singles.tile([128, COLS], F32)
        out_sb = singles.tile([128, COLS], F32)
        rr_tot = singles.tile([128, N_ST // 2 + 1], F32)
        rr1 = singles.tile([128, 1], F32)
        rr_red = singles.tile([128, 1], F32)
        ones_sb = singles.tile([128, 1], F32)
        ones_row = singles.tile([1, 128], F32)
        tiny = singles.tile([128, 1], F32)
        g2 = singles.tile([1, 2], F32)
        cg_sb = singles.tile([1, 2], F32)
        sg1 = singles.tile([1, 2], F32)
        s_g1 = singles.tile([1, 1], F32)
        inv1 = singles.tile([1, 1], F32)
        inv_g = singles.tile([128, 1], F32)

        nc.vector.memset(ones_sb[:], 1.0)
        nc.vector.memset(ones_row[:], 1.0)
        # rr_tot's last column is only written on 96 partitions.
        nc.vector.memset(rr_tot[:], 0.0)
        # Zero the unused corner of E (last super-tile has 96 blocks).
        nc.vector.memset(e_sb[96:128, (N_ST - 1) * Q : N_ST * Q], 0.0)
        # Dummy activation: pulls ACT_TABLE_LOAD off the critical path.
        nc.scalar.activation(
            out=tiny[:], in_=ones_sb[:], func=ACTF.Tanh, bias=0.0, scale=1.0
        )
        if use_coll:
            # Warmup AllReduce: pulls the CC-path cold start off the tail
            # and aligns the cores early.
            warm = singles.tile([1, 2], F32)
            cc_win = nc.dram_tensor("cc_win", [2], F32)
            cc_wout = nc.dram_tensor("cc_wout", [2], F32, addr_space="Shared")
            nc.vector.memset(warm[:], 0.0)
            nc.gpsimd.dma_start(out=cc_win[:], in_=warm[0:1, :])
            nc.gpsimd.collective_compute(
                "AllReduce",
                ALU.add,
                replica_groups=[list(range(n_cores))],
                ins=[cc_win[:]],
                outs=[cc_wout[:]],
            )

        # ---- phase 1: mm1 / tanh / mm2 / exp ---------------------------
        # Global mm1 index m: chunk ch=m//16, j=m%16 -> half=j%2, t=j//2.
        ph = None
        next_blk = 0
        MM2_DEFER = 2  # ht tiles between tanh and its mm2 emission

        # mm2 blocks of a DVE-assisted tile are emitted ASSIST_DEFER tiles
        # late (so the in-order PE stream never waits on the DVE tanh
        # chain).  PSUM accumulate flags: start on the FIRST-emitted block
        # of each 32-strip group (clears that quadrant region only --
        # region-scoped, verified on HW), stop on the last-emitted.
        N_GRP = N_BLK // 32  # 31 (exact)
        grp_cnt = [0] * N_GRP
        pair_tiles = {}
        pair_cnt = {}
        PAIR_LAST = (N_BLK - 1) // 256  # 3
        PAIR_SIZE = [min(N_BLK - 256 * P, 256) for P in range(PAIR_LAST + 1)]

        def emit_exp(P):
            sp = pair_tiles[P]
            nfull = min(PAIR_SIZE[P], 128)
            ntail = PAIR_SIZE[P] - 128
            if ntail == 128 or ntail <= 0:
                ncl = 2 * Q if ntail == 128 else Q
                nc.scalar.activation(
                    out=e_sb[0:nfull, P * 2 * Q : P * 2 * Q + ncl],
                    in_=sp[0:nfull, 0:ncl],
                    func=ACTF.Exp,
                    bias=b2_sb[0:nfull, :],
                    scale=1.0,
                    accum_out=rr_tot[0:nfull, P : P + 1],
                )
            else:
                nc.scalar.activation(
                    out=e_sb[:, P * 2 * Q : P * 2 * Q + Q],
                    in_=sp[:, 0:Q],
                    func=ACTF.Exp,
                    bias=b2_sb[:],
                    scale=1.0,
                    accum_out=rr_tot[:, P : P + 1],
                )
                nc.scalar.activation(
                    out=e_sb[0:ntail, P * 2 * Q + Q : (P + 1) * 2 * Q],
                    in_=sp[0:ntail, Q : 2 * Q],
                    func=ACTF.Exp,
                    bias=b2_sb[0:ntail, :],
                    scale=1.0,
                    accum_out=rr_tot[0:ntail, PAIR_LAST + 1 : PAIR_LAST + 2],
                )
            del pair_tiles[P]

        def emit_block(b, lhs=None):
            c = b % 32
            g = (b % 128) // 32
            gid = b // 32
            P = b // 256
            if P not in pair_tiles:
                pair_tiles[P] = ps_pool.tile(
                    [128, 2 * Q], F32, tag="score", name="score"
                )
            cb = ((b % 256) // 128) * Q
            k = b // BLK_PER_HT
            hoff = Q * (b % BLK_PER_HT)
            nc.tensor.matmul(
                pair_tiles[P][32 * g : 32 * g + 32, cb : cb + Q],
                strips[:, c, :] if lhs is None else lhs,
                ht_tiles[k][:, hoff : hoff + Q],
                start=(grp_cnt[gid] == 0),
                stop=(grp_cnt[gid] == 31),
                skip_group_check=True,
                tile_position=(0, 32 * g),
            )
            grp_cnt[gid] += 1
            pair_cnt[P] = pair_cnt.get(P, 0) + 1
            if pair_cnt[P] == PAIR_SIZE[P]:
                emit_exp(P)

        deferred = []  # (flush_tile, block) in block order
        gate_tiles = {}  # assist tile k -> (gate tile, c0)

        def flush_blocks(j):
            # The scheduler pops ready deferred blocks at any PE-idle
            # moment (its DVE chain estimate is optimistic, and
            # tile_wait_until has no effect in this path), so gate them on
            # an ACT-produced copy of their strip weights: the ACT queue
            # is saturated and priority-dense, so the copy executes at its
            # emission slot (after tanh j-1) -- a true cross-engine clock.
            while deferred and deferred[0][0] <= j:
                f, b = deferred.pop(0)
                k = b // BLK_PER_HT
                if k not in gate_tiles:
                    c0 = (BLK_PER_HT * k) % 32
                    gt = gate_pool.tile([128, 6, 32], F16, tag="gate", name="gt")
                    n1 = min(6, 32 - c0)
                    nc.scalar.activation(
                        out=gt[:, 0:n1, :], in_=strips[:, c0 : c0 + n1, :],
                        func=ACTF.Copy, bias=0.0, scale=1.0,
                    )
                    if n1 < 6:
                        nc.scalar.activation(
                            out=gt[:, n1:6, :], in_=strips[:, 0 : 6 - n1, :],
                            func=ACTF.Copy, bias=0.0, scale=1.0,
                        )
                    gate_tiles[k] = (gt, c0)
                gt, c0 = gate_tiles[k]
                idx = (b % 32 - c0) % 32
                emit_block(b, lhs=gt[:, idx, :])

        def emit_mm2_upto(bmax):
            nonlocal next_blk
            while next_blk < bmax:
                k = next_blk // BLK_PER_HT
                if _is_assist(k):
                    # spread the deferred flush 2 blocks/tile (PE slack is
                    # only ~0.25us/tile -- a 6-block lump starves ACT)
                    off = (next_blk - BLK_PER_HT * k) // 2
                    deferred.append((k + ASSIST_DEFER + off, next_blk))
                else:
                    emit_block(next_blk)
                next_blk += 1

        ht_tiles = {}
        for m in range(N_MM):
            ch, j = divmod(m, MM_PER_CH)
            if j == 4 and ch + 2 < N_CH and (ch + 2) not in xx_tiles:
                ch2 = ch + 2
                q = nc.sync if ch2 % 2 == 0 else nc.scalar
                xx_tiles[ch2] = xx_pool.tile([128, 4096], F16, tag="xx", name="xx")
                q.dma_start(
                    out=xx_tiles[ch2][:],
                    in_=_ap(x_in[:], ch2 * 128 * 4096, [[4096, 128], [1, 4096]]),
                )
            if j == 0 and ch not in xx_tiles:
                q = nc.sync if ch % 2 == 0 else nc.scalar
                xx_tiles[ch] = xx_pool.tile([128, 4096], F16, tag="xx", name="xx")
                q.dma_start(
                    out=xx_tiles[ch][:],
                    in_=_ap(x_in[:], ch * 128 * 4096, [[4096, 128], [1, 4096]]),
                )
            half, t = j % 2, j // 2
            slot = m % PH_MMS
            if slot == 0:
                ph = ph_pool.tile([128, PH_COLS], F32, tag="ph")
            nc.tensor.matmul(
                ph[:, 512 * slot : 512 * (slot + 1)],
                w1t_sb[64 * half : 64 * half + 64, :],
                xx_tiles[ch][64 * half : 64 * half + 64, 512 * t : 512 * (t + 1)],
                start=True,
                stop=True,
            )
            if slot == PH_MMS - 1 or m == N_MM - 1:
                k = m // PH_MMS
                ncols = 512 * (slot + 1)
                ht_tiles[k] = ht_pool.tile([128, PH_COLS], F16, tag="ht", name="ht")
                hk = ht_tiles[k]
                if _is_assist(k) and ncols == PH_COLS:
                    # DVE rational tanh: z = ph + b1; t = z^2; t4 = t^2;
                    # ht = z*(t4 + A2 t + A0) / (t4 + B2 t + B0)
                    zs = dv_pool.tile([128, PH_COLS], F16, tag="dv_zs", name="zs")
                    tq = dv_pool.tile([128, PH_COLS], F16, tag="dv_t", name="tq")
                    t4 = dv_pool.tile([128, PH_COLS], F16, tag="dv_t4", name="t4")
                    nm0 = dv_pool.tile([128, PH_COLS], F16, tag="dv_n0", name="nm0")
                    nm1 = dv_pool.tile([128, PH_COLS], F16, tag="dv_n1", name="nm1")
                    d0 = dv_pool.tile([128, PH_COLS], F16, tag="dv_d0", name="d0")
                    dd = dv_pool.tile([128, PH_COLS], F32, tag="dv_dd", name="dd")
                    rcp = dv_pool.tile([128, PH_COLS], F32, tag="dv_r", name="rcp")
                    # TS/TT only (no STT): the scheduler's DVE cost model
                    # prices STT at 2x but HW runs it at 1x -- the sim/HW
                    # divergence made the scheduler place deferred mm2s too
                    # early in the PE stream.  TS f16 runs 4x, TT f16 2x on
                    # both, so sim chain length ~= HW chain length (~12.6us).
                    aux = dv_pool.tile([128, PH_COLS], F16, tag="dv_x", name="aux")
                    with nc.allow_low_precision(reason="f16 rational tanh"):
                        nc.vector.tensor_scalar(
                            out=zs[:], in0=ph[:], scalar1=b1_sb[:],
                            scalar2=None, op0=ALU.add,
                        )
                        nc.vector.tensor_mul(tq[:], zs[:], zs[:])
                        nc.vector.tensor_mul(t4[:], tq[:], tq[:])
                        nc.vector.tensor_scalar(
                            out=aux[:], in0=tq[:], scalar1=float(A2),
                            scalar2=None, op0=ALU.mult,
                        )
                        nc.vector.tensor_add(nm0[:], aux[:], t4[:])
                        nc.vector.tensor_scalar(
                            out=aux[:], in0=nm0[:], scalar1=float(A0),
                            scalar2=None, op0=ALU.add,
                        )
                        nc.vector.tensor_mul(nm1[:], aux[:], zs[:])
                        nc.vector.tensor_scalar(
                            out=aux[:], in0=tq[:], scalar1=float(B2),
                            scalar2=None, op0=ALU.mult,
                        )
                        nc.vector.tensor_add(d0[:], aux[:], t4[:])
                        nc.vector.tensor_scalar(
                            out=dd[:], in0=d0[:], scalar1=float(B0),
                            scalar2=None, op0=ALU.add,
                        )
                        nc.vector.reciprocal_approx_fast(out=rcp[:], in_=dd[:])
                        nc.vector.tensor_mul(hk[:], nm1[:], rcp[:])
                else:
                    nc.scalar.activation(
                        out=hk[:, 0:ncols],
                        in_=ph[:, 0:ncols],
                        func=ACTF.Tanh,
                        bias=b1_sb[:],
                        scale=1.0,
                    )
                flush_blocks(k)
                if k >= MM2_DEFER:
                    emit_mm2_upto(((k - MM2_DEFER + 1) * PH_COLS) // Q)
        emit_mm2_upto(N_BLK)
        flush_blocks(10**9)

        # ---- core partial sums + allreduce ------------------------------
        nc.vector.reduce_sum(rr1[:], rr_tot[:], axis=mybir.AxisListType.X)
        ps_rr = ps_pool.tile([128, 2], F32, tag="score")
        nc.tensor.matmul(ps_rr[0:1, 0:1], ones_sb[:], rr1[:], start=True, stop=True)
        nc.scalar.activation(
            out=rr_red[0:1, :], in_=ps_rr[0:1, 0:1], func=ACTF.Copy, bias=0.0,
            scale=1.0,
        )
        # g2 = onehot * S_core - padc2   (padc2 = onehot * n_pad * e_pad)
        nc.vector.scalar_tensor_tensor(
            out=g2[0:1, :],
            in0=oh1_sb[0:1, :],
            scalar=rr_red[0:1, 0:1],
            in1=pc2_sb[0:1, :],
            op0=ALU.mult,
            op1=ALU.subtract,
        )

        if use_coll:
            nc.gpsimd.dma_start(out=cc_in[:], in_=g2[0:1, :])
            nc.gpsimd.collective_compute(
                "AllReduce",
                ALU.add,
                replica_groups=[list(range(n_cores))],
                ins=[cc_in[:]],
                outs=[cc_out[:]],
            )
            nc.scalar.dma_start(out=gs_t[:], in_=cc_out[:])
            nc.sync.dma_start(
                out=cg_sb[:], in_=_ap(cc_out[:], 0, [[2, 1], [1, 2]])
            )
        else:
            nc.sync.dma_start(out=gs_t[:], in_=g2[0:1, :])
            nc.sync.dma_start(
                out=cg_sb[:], in_=_ap(gs_t[:], 0, [[2, 1], [1, 2]])
            )

        # ---- normalize + store -----------------------------------------
        # inv_g = 1 / (my group's global sum), broadcast to 128 partitions
        # via a K=1 ones matmul; quarters on alternating HWDGE queues so
        # stores overlap the next quarter's normalize.
        nc.vector.tensor_mul(sg1[:], cg_sb[:], oh1_sb[:])
        nc.vector.reduce_sum(s_g1[:], sg1[:], axis=mybir.AxisListType.X)
        nc.vector.reciprocal(out=inv1[:], in_=s_g1[:])
        ps_b = ps_pool.tile([128, 2], F32, tag="score")
        nc.tensor.matmul(
            ps_b[:, 0:1], ones_row[0:1, :], inv1[0:1, :], start=True, stop=True
        )
        nc.scalar.activation(
            out=inv_g[:], in_=ps_b[:, 0:1], func=ACTF.Copy, bias=0.0, scale=1.0
        )
        NQ = 4
        H = COLS // NQ
        for i in range(NQ):
            eng = nc.sync if i % 2 == 0 else nc.scalar
            sl = slice(i * H, (i + 1) * H)
            nc.vector.tensor_scalar(
                out=out_sb[:, sl],
                in0=e_sb[:, sl],
                scalar1=inv_g[:],
                scalar2=None,
                op0=ALU.mult,
            )
            eng.dma_start(
                out=_ap(out_t[:], i * H, [[COLS, 128], [1, H]]),
                in_=out_sb[:, sl],
            )

    nc.compile()
    return nc


_NC_CACHE = {}


def _get_nc(key=0):
    if key not in _NC_CACHE:
        _NC_CACHE[key] = build_nc()
    return _NC_CACHE[key]


# Host-side index map (identical for every core): for actual row a in
# [0, R_CORE): position in the device (p, col) layout.
_IDX_CACHE = {}


def _layout_index():
    """Return (p, col) arrays mapping core-row a -> device layout slot."""
    if "idx" not in _IDX_CACHE:
        a = np.arange(R_CORE, dtype=np.int64)
        ch, o = a // CH_ROWS, a % CH_ROWS
        half, w = o // 4096, o % 4096
        t, jr = w // 512, w % 512
        m = MM_PER_CH * ch + 2 * t + half
        q = 512 * m + jr
        b, r = q // Q, q % Q
        p = b % 128
        col = (b // 128) * Q + r
        _IDX_CACHE["idx"] = (p, col)
    return _IDX_CACHE["idx"]


def prep_inputs(x, T, W1, b1, W2, b2, n_cores=N_CORES):
    """Host-side shard/layout prep -> (per-core input maps, perm).

    Rows are stably partitioned by group: cores 0..k0-1 get group-0 rows,
    cores k0..7 group-1, with zero-padding at the two group tails so every
    core is group-pure.
    """
    n_rows = x.shape[0]
    assert n_rows == N_ROWS

    T = np.asarray(T).astype(np.int64).reshape(-1)
    perm = np.argsort(T, kind="stable")
    g0 = int((T == 0).sum())
    g1 = n_rows - g0
    k0 = int(np.ceil(g0 / R_CORE))
    assert k0 * R_CORE >= g0 and (n_cores - k0) * R_CORE >= g1, (
        "group sizes too imbalanced for group-pure sharding"
    )

    x = np.asarray(x, dtype=np.float32)
    xg = x[perm].astype(np.float16)
    xh = np.zeros((N_PAD, IN_DIM), dtype=np.float16)
    xh[:g0] = xg[:g0]
    xh[k0 * R_CORE : k0 * R_CORE + g1] = xg[g0:]
    del xg
    # [n_chunk_tot, 2, 4096, 64] -> [n_chunk_tot, 2, 64, 4096] -> [., 128, 4096]
    n_ch_tot = N_PAD // CH_ROWS
    xd = np.ascontiguousarray(
        xh.reshape(n_ch_tot, 2, 4096, IN_DIM)
        .transpose(0, 1, 3, 2)
        .reshape(n_ch_tot, 128, 4096)
    )

    w1t = np.ascontiguousarray(np.asarray(W1, np.float32).T).astype(np.float16)
    w2s = np.zeros((HID, 32, 32), dtype=np.float16)
    w2v = np.asarray(W2, np.float32).reshape(HID).astype(np.float16)
    for c in range(32):
        w2s[:, c, c] = w2v
    w2s = w2s.reshape(HID, 32 * 32)
    b1h = np.asarray(b1, np.float32).reshape(HID).copy()
    b2h = np.asarray(b2, np.float32).reshape(1).copy()

    # Host estimate of the device's E value on padded rows (x = 0).
    h_pad = np.tanh(b1h)
    e_pad = float(np.exp(b2h[0] + np.asarray(W2, np.float32).reshape(HID) @ h_pad))

    in_maps = []
    for cid in range(n_cores):
        grp = 0 if cid < k0 else 1
        if cid < k0:
            n_real = min(max(g0 - cid * R_CORE, 0), R_CORE)
        else:
            n_real = min(max(g1 - (cid - k0) * R_CORE, 0), R_CORE)
        n_pad_c = R_CORE - n_real
        oh = np.zeros(2, dtype=np.float32)
        oh[grp] = 1.0
        padc2 = oh * (n_pad_c * e_pad)
        in_maps.append(
            {
                "x": xd[cid * N_CH : (cid + 1) * N_CH],
                "w1t": w1t,
                "w2s": w2s,
                "b1": b1h,
                "b2": b2h,
                "oh": oh,
                "padc2": padc2,
            }
        )
    return in_maps, perm, g0, k0


def run(x, T, W1, b1, W2, b2, trace=False):
    in_maps, perm, g0, k0 = prep_inputs(x, T, W1, b1, W2, b2)
    nc = _get_nc()
    res = run_bass_kernel_spmd(nc, in_maps, list(range(N_CORES)), trace=trace)
    p_idx, c_idx = _layout_index()
    n_rows = x.shape[0]
    g1 = n_rows - g0
    # Collect per-core real rows (device layout -> permuted row order).
    vals = np.empty(N_PAD, dtype=np.float32)
    for cid in range(N_CORES):
        lay = res.results[cid]["out"].reshape(128, COLS)
        vals[cid * R_CORE : (cid + 1) * R_CORE] = lay[p_idx, c_idx]
    out = np.empty(n_rows, dtype=np.float32)
    out[perm[:g0]] = vals[:g0]
    out[perm[g0:]] = vals[k0 * R_CORE : k0 * R_CORE + g1]
    return out, res


def kernel(x, T, W1, b1, W2, b2):
    out, _ = run(x, T, W1, b1, W2, b2)
    return out


# revision 24
# speedup vs baseline: 1.1742x; 1.1742x over previous
"""
Trainium2 Bass kernel for DirectRankingModel:
    h = tanh(x @ W1.T + b1); s = (h @ W2.T + b2); e = exp(s)
    out = e / segment_sum(e, T)[T]    with 2 segments, N = 2,000,000 rows.

Strategy (8 NeuronCores, v3):
  - Host: rows are PARTITIONED BY GROUP across cores (stable sort by T):
    cores 0..k0-1 hold only group-0 rows, cores k0..7 only group-1 (k0=4
    for balanced random T).  The group boundary is padded to a core
    boundary (zero rows, corrected via a host-computed scalar), so every
    core is group-pure: no sel mask, segment sum == plain sum.
  - Host casts x to f16 and block-transposes into chunks of 8192 rows:
    xd[ch] = [128, 4096]; partitions 0-63 hold features of rows [0,4096),
    64-127 rows [4096,8192) -> row-split PE runs both halves concurrently.
  - 31 chunks (253952 rows) per core.
  - mm1: K=64 f16 matmuls, N=512, half-pairs on partition halves -> PSUM
    ph [128, 1536] (3 banks, 2 bufs).
  - tanh on ACT in 1536-wide instructions (PSUM->SBUF f16) -- ACT is the
    kernel's roofline (~0.93 ns/col/core measured).
  - mm2: 32-strip trick; block b -> psum partition b%128, strips cycle
    c=b%32 with col-group tile_position; exp per 128-block super-tile.
  - Sums: per-super-tile DVE reduce of e; ones-matmul partition reduce;
    per-core partial = onehot * S - padc2; 2-float HBM AllReduce.
  - normalize: out = E * (1/sum_of_my_group) -- one tensor_scalar per
    half, store halves on alternating HWDGE queues.
"""

import os
import sys

import numpy as np

for _p in ("/opt/trn_rl_repo", "/root/.axon_site/_ro/trn_rl_repo"):
    if os.path.isdir(_p) and _p not in sys.path:
        sys.path.insert(0, _p)

import concourse.bacc as bacc
import concourse.bass as bass
import concourse.tile as tile
from concourse import mybir
from concourse.bass_utils import run_bass_kernel_spmd

F16 = mybir.dt.float16
F32 = mybir.dt.float32
ALU = mybir.AluOpType
ACTF = mybir.ActivationFunctionType

N_CORES = 8
N_ROWS = 2_000_000
IN_DIM = 64
HID = 128

# Device-side geometry (per core).
Q = 256                     # rows per score-block
CH_ROWS = 8192              # rows per DMA chunk ([128, 4096] f16 = 1 MiB)
N_CH = 31                   # chunks per core
R_CORE = N_CH * CH_ROWS     # 253952 rows per core
N_PAD = N_CORES * R_CORE    # 2031616 rows total (padded)
MM_PER_CH = 16              # mm1 matmuls (512 rows each) per chunk
N_MM = N_CH * MM_PER_CH     # 496
PH_MMS = 3                  # mm1 outputs per PSUM tile -> ACT N=1536
PH_COLS = 512 * PH_MMS
BLK_PER_HT = PH_COLS // Q   # 6 score blocks per ht tile
N_BLK = R_CORE // Q         # 992
N_ST = (N_BLK + 127) // 128  # 8 super-tiles (last partial: 96 blocks)
COLS = N_ST * Q             # 2048 e/out columns per partition


def _ap(handle_ap, offset, dims):
    """Custom access pattern on a DRAM tensor: dims = [[step, count], ...]."""
    return bass.AP(tensor=handle_ap.tensor, offset=offset, ap=list(dims))


# DVE-assisted tanh: every ASSIST_EVERY-th full ht tile is computed on the
# vector engine with a minimax R(5,4) rational instead of ACT (the kernel's
# bottleneck): tanh(z) ~ z*(t^2 + A2 t + A0)/(t^2 + B2 t + B0), t = z^2,
# fit on |z| <= 4.8; f16 end-to-end max err ~2.7e-3.  The assisted tile's
# mm2 strip-groups are deferred ASSIST_DEFER tiles (whole 32-block groups,
# emitted atomically, so PSUM start/accumulate flags stay well-formed) to
# keep the in-order PE stream from ever waiting on the DVE chain.
ASSIST_EVERY = 0            # 0 disables (DVE tanh assist measured a net wash)
ASSIST_START = 8
ASSIST_STOP = 148
ASSIST_DEFER = 12
A2, A0 = -97.93266143005741, -1801.597436686564
B2, B0 = -680.3934051370769, -1811.5100012510918


def _is_assist(k):
    return (
        ASSIST_EVERY > 0
        and ASSIST_START <= k < ASSIST_STOP
        and (k - ASSIST_START) % ASSIST_EVERY == 0
    )


def build_nc(n_cores=N_CORES, use_coll=True):
    """Build the per-core Bass program (SPMD: same program, sliced inputs)."""
    from contextlib import ExitStack

    nc = bacc.Bacc(num_devices=n_cores)

    x_in = nc.declare_dram_parameter("x", [N_CH, 128, 4096], F16, isOutput=False)
    w1t_in = nc.declare_dram_parameter("w1t", [IN_DIM, HID], F16, isOutput=False)
    w2s_in = nc.declare_dram_parameter("w2s", [HID, 32 * 32], F16, isOutput=False)
    b1_in = nc.declare_dram_parameter("b1", [HID], F32, isOutput=False)
    b2_in = nc.declare_dram_parameter("b2", [1], F32, isOutput=False)
    oh_in = nc.declare_dram_parameter("oh", [2], F32, isOutput=False)
    pc2_in = nc.declare_dram_parameter("padc2", [2], F32, isOutput=False)
    out_t = nc.declare_dram_parameter("out", [128 * COLS], F32, isOutput=True)
    gs_t = nc.declare_dram_parameter("gsums", [2], F32, isOutput=True)

    cc_in = nc.dram_tensor("cc_in", [2], F32)
    cc_out = nc.dram_tensor("cc_out", [2], F32, addr_space="Shared")

    with ExitStack() as ctx:
        tc = ctx.enter_context(tile.TileContext(nc))
        singles = ctx.enter_context(tc.tile_pool(name="singles", bufs=1))
        xx_pool = ctx.enter_context(tc.tile_pool(name="xx", bufs=3))
        ht_pool = ctx.enter_context(
            tc.tile_pool(name="ht", bufs=(ASSIST_DEFER + 7) if ASSIST_EVERY else 6)
        )
        dv_pool = ctx.enter_context(tc.tile_pool(name="dv", bufs=1))
        gate_pool = ctx.enter_context(tc.tile_pool(name="gate", bufs=2))
        ph_pool = ctx.enter_context(tc.tile_pool(name="ph", bufs=2, space="PSUM"))
        ps_pool = ctx.enter_context(tc.tile_pool(name="ps", bufs=2, space="PSUM"))

        # ---- static setup ----------------------------------------------
        # First x chunk before anything else on the sync HWDGE queue; a
        # small leading slice so the first matmul can start earlier.
        xx_tiles = {}
        xx_tiles[0] = xx_pool.tile([128, 4096], F16, tag="xx", name="xx")
        w1t_sb = singles.tile([128, HID], F16)     # both halves hold W1T
        nc.sync.dma_start(
            out=w1t_sb[:],
            in_=_ap(w1t_in[:], 0, [[0, 2], [HID, IN_DIM], [1, HID]]),
        )
        # Fast-start slices: ph tile 0's mm1 reads (cols 0:1024), split
        # across both HWDGE queues.
        nc.sync.dma_start(
            out=xx_tiles[0][:, 0:512],
            in_=_ap(x_in[:], 0, [[4096, 128], [1, 512]]),
        )
        b1_sb = singles.tile([128, 1], F32)
        nc.sync.dma_start(out=b1_sb[:], in_=_ap(b1_in[:], 0, [[1, HID], [1, 1]]))
        b2_sb = singles.tile([128, 1], F32)
        nc.sync.dma_start(out=b2_sb[:], in_=_ap(b2_in[:], 0, [[0, 128], [1, 1]]))
        xx_tiles[1] = xx_pool.tile([128, 4096], F16, tag="xx", name="xx")
        nc.sync.dma_start(
            out=xx_tiles[1][:],
            in_=_ap(x_in[:], 128 * 4096, [[4096, 128], [1, 4096]]),
        )

        # Scalar HWDGE queue: rest of chunk 0, strips, small params.
        nc.scalar.dma_start(
            out=xx_tiles[0][:, 512:1024],
            in_=_ap(x_in[:], 512, [[4096, 128], [1, 512]]),
        )
        # 32 strip matrices [128, 32] fp16, strip c has W2 in column c.
        strips = singles.tile([128, 32, 32], F16)
        nc.scalar.dma_start(
            out=strips[:], in_=_ap(w2s_in[:], 0, [[32 * 32, HID], [1, 32 * 32]])
        )
        nc.scalar.dma_start(
            out=xx_tiles[0][:, 1024:4096],
            in_=_ap(x_in[:], 1024, [[4096, 128], [1, 3072]]),
        )
        oh1_sb = singles.tile([1, 2], F32)
        nc.scalar.dma_start(out=oh1_sb[:], in_=_ap(oh_in[:], 0, [[2, 1], [1, 2]]))
        pc2_sb = singles.tile([1, 2], F32)
        nc.scalar.dma_start(out=pc2_sb[:], in_=_ap(pc2_in[:], 0, [[2, 1], [1, 2]]))

        e_sb = singles.tile([128, COLS], F32)
        out_sb = singles.tile([128, COLS], F32)
        rr_tot = singles.tile([128, N_ST // 2 + 1], F32)
        rr1 = singles.tile([128, 1], F32)
        rr_red = singles.tile([128, 1], F32)
        ones_sb = singles.tile([128, 1], F32)
        ones_row = singles.tile([1, 128], F32)
        tiny = singles.tile([128, 1], F32)
        g2 = singles.tile([1, 2], F32)
        cg_sb = singles.tile([1, 2], F32)
        sg1 = singles.tile([1, 2], F32)
        s_g1 = singles.tile([1, 1], F32)
        inv1 = singles.tile([1, 1], F32)
        inv_g = singles.tile([128, 1], F32)

        nc.vector.memset(ones_sb[:], 1.0)
        nc.vector.memset(ones_row[:], 1.0)
        # rr_tot's last column is only written on 96 partitions.
        nc.vector.memset(rr_tot[:], 0.0)
        # Zero the unused corner of E (last super-tile has 96 blocks).
        nc.vector.memset(e_sb[96:128, (N_ST - 1) * Q : N_ST * Q], 0.0)
        # Dummy activation: pulls ACT_TABLE_LOAD off the critical path.
        nc.scalar.activation(
            out=tiny[:], in_=ones_sb[:], func=ACTF.Tanh, bias=0.0, scale=1.0
        )
        if use_coll:
            # Warmup AllReduce: pulls the CC-path cold start off the tail
            # and aligns the cores early.
            warm = singles.tile([1, 2], F32)
            cc_win = nc.dram_tensor("cc_win", [2], F32)
            cc_wout = nc.dram_tensor("cc_wout", [2], F32, addr_space="Shared")
            nc.vector.memset(warm[:], 0.0)
            nc.gpsimd.dma_start(out=cc_win[:], in_=warm[0:1, :])
            nc.gpsimd.collective_compute(
                "AllReduce",
                ALU.add,
                replica_groups=[list(range(n_cores))],
                ins=[cc_win[:]],
                outs=[cc_wout[:]],
            )

        # ---- phase 1: mm1 / tanh / mm2 / exp ---------------------------
        # Global mm1 index m: chunk ch=m//16, j=m%16 -> half=j%2, t=j//2.
        ph = None
        next_blk = 0
        MM2_DEFER = 2  # ht tiles between tanh and its mm2 emission

        # mm2 blocks of a DVE-assisted tile are emitted ASSIST_DEFER tiles
        # late (so the in-order PE stream never waits on the DVE tanh
        # chain).  PSUM accumulate flags: start on the FIRST-emitted block
        # of each 32-strip group (clears that quadrant region only --
        # region-scoped, verified on HW), stop on the last-emitted.
        N_GRP = N_BLK // 32  # 31 (exact)
        grp_cnt = [0] * N_GRP
        pair_tiles = {}
        pair_cnt = {}
        PAIR_LAST = (N_BLK - 1) // 256  # 3
        PAIR_SIZE = [min(N_BLK - 256 * P, 256) for P in range(PAIR_LAST + 1)]

        def emit_exp(P):
            sp = pair_tiles[P]
            nfull = min(PAIR_SIZE[P], 128)
            ntail = PAIR_SIZE[P] - 128
            if ntail == 128 or ntail <= 0:
                ncl = 2 * Q if ntail == 128 else Q
                nc.scalar.activation(
                    out=e_sb[0:nfull, P * 2 * Q : P * 2 * Q + ncl],
                    in_=sp[0:nfull, 0:ncl],
                    func=ACTF.Exp,
                    bias=b2_sb[0:nfull, :],
                    scale=1.0,
                    accum_out=rr_tot[0:nfull, P : P + 1],
                )
            else:
                nc.scalar.activation(
                    out=e_sb[:, P * 2 * Q : P * 2 * Q + Q],
                    in_=sp[:, 0:Q],
                    func=ACTF.Exp,
                    bias=b2_sb[:],
                    scale=1.0,
                    accum_out=rr_tot[:, P : P + 1],
                )
                nc.scalar.activation(
                    out=e_sb[0:ntail, P * 2 * Q + Q : (P + 1) * 2 * Q],
                    in_=sp[0:ntail, Q : 2 * Q],
                    func=ACTF.Exp,
                    bias=b2_sb[0:ntail, :],
                    scale=1.0,
                    accum_out=rr_tot[0:ntail, PAIR_LAST + 1 : PAIR_LAST + 2],
                )
            del pair_tiles[P]

        def emit_block(b, lhs=None):
            c = b % 32
            g = (b % 128) // 32
            gid = b // 32
            P = b // 256
            if P not in pair_tiles:
                pair_tiles[P] = ps_pool.tile(
                    [128, 2 * Q], F32, tag="score", name="score"
                )
            cb = ((b % 256) // 128) * Q
            k = b // BLK_PER_HT
            hoff = Q * (b % BLK_PER_HT)
            nc.tensor.matmul(
                pair_tiles[P][32 * g : 32 * g + 32, cb : cb + Q],
                strips[:, c, :] if lhs is None else lhs,
                ht_tiles[k][:, hoff : hoff + Q],
                start=(grp_cnt[gid] == 0),
                stop=(grp_cnt[gid] == 31),
                skip_group_check=True,
                tile_position=(0, 32 * g),
            )
            grp_cnt[gid] += 1
            pair_cnt[P] = pair_cnt.get(P, 0) + 1
            if pair_cnt[P] == PAIR_SIZE[P]:
                emit_exp(P)

        deferred = []  # (flush_tile, block) in block order
        gate_tiles = {}  # assist tile k -> (gate tile, c0)

        def flush_blocks(j):
            # The scheduler pops ready deferred blocks at any PE-idle
            # moment (its DVE chain estimate is optimistic, and
            # tile_wait_until has no effect in this path), so gate them on
            # an ACT-produced copy of their strip weights: the ACT queue
            # is saturated and priority-dense, so the copy executes at its
            # emission slot (after tanh j-1) -- a true cross-engine clock.
            while deferred and deferred[0][0] <= j:
                f, b = deferred.pop(0)
                k = b // BLK_PER_HT
                if k not in gate_tiles:
                    c0 = (BLK_PER_HT * k) % 32
                    gt = gate_pool.tile([128, 6, 32], F16, tag="gate", name="gt")
                    n1 = min(6, 32 - c0)
                    nc.scalar.activation(
                        out=gt[:, 0:n1, :], in_=strips[:, c0 : c0 + n1, :],
                        func=ACTF.Copy, bias=0.0, scale=1.0,
                    )
                    if n1 < 6:
                        nc.scalar.activation(
                            out=gt[:, n1:6, :], in_=strips[:, 0 : 6 - n1, :],
                            func=ACTF.Copy, bias=0.0, scale=1.0,
                        )
                    gate_tiles[k] = (gt, c0)
                gt, c0 = gate_tiles[k]
                idx = (b % 32 - c0) % 32
                emit_block(b, lhs=gt[:, idx, :])

        def emit_mm2_upto(bmax):
            nonlocal next_blk
            while next_blk < bmax:
                k = next_blk // BLK_PER_HT
                if _is_assist(k):
                    # spread the deferred flush 2 blocks/tile (PE slack is
                    # only ~0.25us/tile -- a 6-block lump starves ACT)
                    off = (next_blk - BLK_PER_HT * k) // 2
                    deferred.append((k + ASSIST_DEFER + off, next_blk))
                else:
                    emit_block(next_blk)
                next_blk += 1

        ht_tiles = {}
        for m in range(N_MM):
            ch, j = divmod(m, MM_PER_CH)
            if j == 4 and ch + 2 < N_CH and (ch + 2) not in xx_tiles:
                ch2 = ch + 2
                q = nc.sync if ch2 % 2 == 0 else nc.scalar
                xx_tiles[ch2] = xx_pool.tile([128, 4096], F16, tag="xx", name="xx")
                q.dma_start(
                    out=xx_tiles[ch2][:],
                    in_=_ap(x_in[:], ch2 * 128 * 4096, [[4096, 128], [1, 4096]]),
                )
            if j == 0 and ch not in xx_tiles:
                q = nc.sync if ch % 2 == 0 else nc.scalar
                xx_tiles[ch] = xx_pool.tile([128, 4096], F16, tag="xx", name="xx")
                q.dma_start(
                    out=xx_tiles[ch][:],
                    in_=_ap(x_in[:], ch * 128 * 4096, [[4096, 128], [1, 4096]]),
                )
            half, t = j % 2, j // 2
            slot = m % PH_MMS
            if slot == 0:
                ph = ph_pool.tile([128, PH_COLS], F32, tag="ph")
            nc.tensor.matmul(
                ph[:, 512 * slot : 512 * (slot + 1)],
                w1t_sb[64 * half : 64 * half + 64, :],
                xx_tiles[ch][64 * half : 64 * half + 64, 512 * t : 512 * (t + 1)],
                start=True,
                stop=True,
            )
            if slot == PH_MMS - 1 or m == N_MM - 1:
                k = m // PH_MMS
                ncols = 512 * (slot + 1)
                ht_tiles[k] = ht_pool.tile([128, PH_COLS], F16, tag="ht", name="ht")
                hk = ht_tiles[k]
                if _is_assist(k) and ncols == PH_COLS:
                    # DVE rational tanh: z = ph + b1; t = z^2; t4 = t^2;
                    # ht = z*(t4 + A2 t + A0) / (t4 + B2 t + B0)
                    zs = dv_pool.tile([128, PH_COLS], F16, tag="dv_zs", name="zs")
                    tq = dv_pool.tile([128, PH_COLS], F16, tag="dv_t", name="tq")
                    t4 = dv_pool.tile([128, PH_COLS], F16, tag="dv_t4", name="t4")
                    nm0 = dv_pool.tile([128, PH_COLS], F16, tag="dv_n0", name="nm0")
                    nm1 = dv_pool.tile([128, PH_COLS], F16, tag="dv_n1", name="nm1")
                    d0 = dv_pool.tile([128, PH_COLS], F16, tag="dv_d0", name="d0")
                    dd = dv_pool.tile([128, PH_COLS], F32, tag="dv_dd", name="dd")
                    rcp = dv_pool.tile([128, PH_COLS], F32, tag="dv_r", name="rcp")
                    # TS/TT only (no STT): the scheduler's DVE cost model
                    # prices STT at 2x but HW runs it at 1x -- the sim/HW
                    # divergence made the scheduler place deferred mm2s too
                    # early in the PE stream.  TS f16 runs 4x, TT f16 2x on
                    # both, so sim chain length ~= HW chain length (~12.6us).
                    aux = dv_pool.tile([128, PH_COLS], F16, tag="dv_x", name="aux")
                    with nc.allow_low_precision(reason="f16 rational tanh"):
                        nc.vector.tensor_scalar(
                            out=zs[:], in0=ph[:], scalar1=b1_sb[:],
                            scalar2=None, op0=ALU.add,
                        )
                        nc.vector.tensor_mul(tq[:], zs[:], zs[:])
                        nc.vector.tensor_mul(t4[:], tq[:], tq[:])
                        nc.vector.tensor_scalar(
                            out=aux[:], in0=tq[:], scalar1=float(A2),
                            scalar2=None, op0=ALU.mult,
                        )
                        nc.vector.tensor_add(nm0[:], aux[:], t4[:])
                        nc.vector.tensor_scalar(
                            out=aux[:], in0=nm0[:], scalar1=float(A0),
                            scalar2=None, op0=ALU.add,
                        )
                        nc.vector.tensor_mul(nm1[:], aux[:], zs[:])
                        nc.vector.tensor_scalar(
                            out=aux[:], in0=tq[:], scalar1=float(B2),
                            scalar2=None, op0=ALU.mult,
                        )
                        nc.vector.tensor_add(d0[:], aux[:], t4[:])
                        nc.vector.tensor_scalar(
                            out=dd[:], in0=d0[:], scalar1=float(B0),
                            scalar2=None, op0=ALU.add,
                        )
                        nc.vector.reciprocal_approx_fast(out=rcp[:], in_=dd[:])
                        nc.vector.tensor_mul(hk[:], nm1[:], rcp[:])
                else:
                    nc.scalar.activation(
                        out=hk[:, 0:ncols],
                        in_=ph[:, 0:ncols],
                        func=ACTF.Tanh,
                        bias=b1_sb[:],
                        scale=1.0,
                    )
                flush_blocks(k)
                if k >= MM2_DEFER:
                    emit_mm2_upto(((k - MM2_DEFER + 1) * PH_COLS) // Q)
        emit_mm2_upto(N_BLK)
        flush_blocks(10**9)

        # ---- core partial sums + allreduce ------------------------------
        nc.vector.reduce_sum(rr1[:], rr_tot[:], axis=mybir.AxisListType.X)
        ps_rr = ps_pool.tile([128, 2], F32, tag="score")
        nc.tensor.matmul(ps_rr[0:1, 0:1], ones_sb[:], rr1[:], start=True, stop=True)
        nc.scalar.activation(
            out=rr_red[0:1, :], in_=ps_rr[0:1, 0:1], func=ACTF.Copy, bias=0.0,
            scale=1.0,
        )
        # g2 = onehot * S_core - padc2   (padc2 = onehot * n_pad * e_pad)
        nc.vector.scalar_tensor_tensor(
            out=g2[0:1, :],
            in0=oh1_sb[0:1, :],
            scalar=rr_red[0:1, 0:1],
            in1=pc2_sb[0:1, :],
            op0=ALU.mult,
            op1=ALU.subtract,
        )

        if use_coll:
            nc.gpsimd.dma_start(out=cc_in[:], in_=g2[0:1, :])
            nc.gpsimd.collective_compute(
                "AllReduce",
                ALU.add,
                replica_groups=[list(range(n_cores))],
                ins=[cc_in[:]],
                outs=[cc_out[:]],
            )
            nc.scalar.dma_start(out=gs_t[:], in_=cc_out[:])
            nc.sync.dma_start(
                out=cg_sb[:], in_=_ap(cc_out[:], 0, [[2, 1], [1, 2]])
            )
        else:
            nc.sync.dma_start(out=gs_t[:], in_=g2[0:1, :])
            nc.sync.dma_start(
                out=cg_sb[:], in_=_ap(gs_t[:], 0, [[2, 1], [1, 2]])
            )

        # ---- normalize + store -----------------------------------------
        # inv_g = 1 / (my group's global sum), broadcast to 128 partitions
        # via a K=1 ones matmul; quarters on alternating HWDGE queues so
        # stores overlap the next quarter's normalize.
        nc.vector.tensor_mul(sg1[:], cg_sb[:], oh1_sb[:])
        nc.vector.reduce_sum(s_g1[:], sg1[:], axis=mybir.AxisListType.X)
        nc.vector.reciprocal(out=inv1[:], in_=s_g1[:])
        ps_b = ps_pool.tile([128, 2], F32, tag="score")
        nc.tensor.matmul(
            ps_b[:, 0:1], ones_row[0:1, :], inv1[0:1, :], start=True, stop=True
        )
        nc.scalar.activation(
            out=inv_g[:], in_=ps_b[:, 0:1], func=ACTF.Copy, bias=0.0, scale=1.0
        )
        NQ = 4
        H = COLS // NQ
        for i in range(NQ):
            eng = nc.sync if i % 2 == 0 else nc.scalar
            sl = slice(i * H, (i + 1) * H)
            nc.vector.tensor_scalar(
                out=out_sb[:, sl],
                in0=e_sb[:, sl],
                scalar1=inv_g[:],
                scalar2=None,
                op0=ALU.mult,
            )
            eng.dma_start(
                out=_ap(out_t[:], i * H, [[COLS, 128], [1, H]]),
                in_=out_sb[:, sl],
            )

    nc.compile()
    return nc


_NC_CACHE = {}


def _get_nc(key=0):
    if key not in _NC_CACHE:
        _NC_CACHE[key] = build_nc()
    return _NC_CACHE[key]


# Host-side index map (identical for every core): for actual row a in
# [0, R_CORE): position in the device (p, col) layout.
_IDX_CACHE = {}


def _layout_index():
    """Return (p, col) arrays mapping core-row a -> device layout slot."""
    if "idx" not in _IDX_CACHE:
        a = np.arange(R_CORE, dtype=np.int64)
        ch, o = a // CH_ROWS, a % CH_ROWS
        half, w = o // 4096, o % 4096
        t, jr = w // 512, w % 512
        m = MM_PER_CH * ch + 2 * t + half
        q = 512 * m + jr
        b, r = q // Q, q % Q
        p = b % 128
        col = (b // 128) * Q + r
        _IDX_CACHE["idx"] = (p, col)
    return _IDX_CACHE["idx"]


def prep_inputs(x, T, W1, b1, W2, b2, n_cores=N_CORES):
    """Host-side shard/layout prep -> (per-core input maps, perm).

    Rows are stably partitioned by group: cores 0..k0-1 get group-0 rows,
    cores k0..7 group-1, with zero-padding at the two group tails so every
    core is group-pure.
    """
    n_rows = x.shape[0]
    assert n_rows == N_ROWS

    T = np.asarray(T).astype(np.int64).reshape(-1)
    perm = np.argsort(T, kind="stable")
    g0 = int((T == 0).sum())
    g1 = n_rows - g0
    k0 = int(np.ceil(g0 / R_CORE))
    assert k0 * R_CORE >= g0 and (n_cores - k0) * R_CORE >= g1, (
        "group sizes too imbalanced for group-pure sharding"
    )

    x = np.asarray(x, dtype=np.float32)
    xg = x[perm].astype(np.float16)
    xh = np.zeros((N_PAD, IN_DIM), dtype=np.float16)
    xh[:g0] = xg[:g0]
    xh[k0 * R_CORE : k0 * R_CORE + g1] = xg[g0:]
    del xg
    # [n_chunk_tot, 2, 4096, 64] -> [n_chunk_tot, 2, 64, 4096] -> [., 128, 4096]
    n_ch_tot = N_PAD // CH_ROWS
    xd = np.ascontiguousarray(
        xh.reshape(n_ch_tot, 2, 4096, IN_DIM)
        .transpose(0, 1, 3, 2)
        .reshape(n_ch_tot, 128, 4096)
    )

    w1t = np.ascontiguousarray(np.asarray(W1, np.float32).T).astype(np.float16)
    w2s = np.zeros((HID, 32, 32), dtype=np.float16)
    w2v = np.asarray(W2, np.float32).reshape(HID).astype(np.float16)
    for c in range(32):
        w2s[:, c, c] = w2v
    w2s = w2s.reshape(HID, 32 * 32)
    b1h = np.asarray(b1, np.float32).reshape(HID).copy()
    b2h = np.asarray(b2, np.float32).reshape(1).copy()

    # Host estimate of the device's E value on padded rows (x = 0).
    h_pad = np.tanh(b1h)
    e_pad = float(np.exp(b2h[0] + np.asarray(W2, np.float32).reshape(HID) @ h_pad))

    in_maps = []
    for cid in range(n_cores):
        grp = 0 if cid < k0 else 1
        if cid < k0:
            n_real = min(max(g0 - cid * R_CORE, 0), R_CORE)
        else:
            n_real = min(max(g1 - (cid - k0) * R_CORE, 0), R_CORE)
        n_pad_c = R_CORE - n_real
        oh = np.zeros(2, dtype=np.float32)
        oh[grp] = 1.0
        padc2 = oh * (n_pad_c * e_pad)
        in_maps.append(
            {
                "x": xd[cid * N_CH : (cid + 1) * N_CH],
                "w1t": w1t,
                "w2s": w2s,
                "b1": b1h,
                "b2": b2h,
                "oh": oh,
                "padc2": padc2,
            }
        )
    return in_maps, perm, g0, k0


def run(x, T, W1, b1, W2, b2, trace=False):
    in_maps, perm, g0, k0 = prep_inputs(x, T, W1, b1, W2, b2)
    nc = _get_nc()
    res = run_bass_kernel_spmd(nc, in_maps, list(range(N_CORES)), trace=trace)
    p_idx, c_idx = _layout_index()
    n_rows = x.shape[0]
    g1 = n_rows - g0
    # Collect per-core real rows (device layout -> permuted row order).
    vals = np.empty(N_PAD, dtype=np.float32)
    for cid in range(N_CORES):
        lay = res.results[cid]["out"].reshape(128, COLS)
        vals[cid * R_CORE : (cid + 1) * R_CORE] = lay[p_idx, c_idx]
    out = np.empty(n_rows, dtype=np.float32)
    out[perm[:g0]] = vals[:g0]
    out[perm[g0:]] = vals[k0 * R_CORE : k0 * R_CORE + g1]
    return out, res


def kernel(x, T, W1, b1, W2, b2):
    out, _ = run(x, T, W1, b1, W2, b2)
    return out


# revision 26
# speedup vs baseline: 1.2125x; 1.0326x over previous
"""
Trainium2 Bass kernel for DirectRankingModel:
    h = tanh(x @ W1.T + b1); s = (h @ W2.T + b2); e = exp(s)
    out = e / segment_sum(e, T)[T]    with 2 segments, N = 2,000,000 rows.

Strategy (8 NeuronCores, v3):
  - Host: rows are PARTITIONED BY GROUP across cores (stable sort by T):
    cores 0..k0-1 hold only group-0 rows, cores k0..7 only group-1 (k0=4
    for balanced random T).  The group boundary is padded to a core
    boundary (zero rows, corrected via a host-computed scalar), so every
    core is group-pure: no sel mask, segment sum == plain sum.
  - Host casts x to f16 and block-transposes into chunks of 8192 rows:
    xd[ch] = [128, 4096]; partitions 0-63 hold features of rows [0,4096),
    64-127 rows [4096,8192) -> row-split PE runs both halves concurrently.
  - 31 chunks (253952 rows) per core.
  - mm1: K=64 f16 matmuls, N=512, half-pairs on partition halves -> PSUM
    ph [128, 1536] (3 banks, 2 bufs).
  - tanh on ACT in 1536-wide instructions (PSUM->SBUF f16) -- ACT is the
    kernel's roofline (~0.93 ns/col/core measured).
  - mm2: 32-strip trick; block b -> psum partition b%128, strips cycle
    c=b%32 with col-group tile_position; exp per 256-block PSUM pair
    ([128,512] = 1 bank) with accum_out giving per-partition sums free.
  - Sums: exp accum_out -> ones-matmul partition reduce; per-core
    partial = onehot * S - padc2; 2-float HBM AllReduce; inverse
    broadcast to 128 partitions via a K=1 ones matmul.
  - normalize: out = E * (1/sum_of_my_group) -- tensor_scalar quarters,
    stores on alternating HWDGE queues.
"""

import os
import sys

import numpy as np

for _p in ("/opt/trn_rl_repo", "/root/.axon_site/_ro/trn_rl_repo"):
    if os.path.isdir(_p) and _p not in sys.path:
        sys.path.insert(0, _p)

import concourse.bacc as bacc
import concourse.bass as bass
import concourse.tile as tile
from concourse import mybir
from concourse.bass_utils import run_bass_kernel_spmd

F16 = mybir.dt.float16
F32 = mybir.dt.float32
ALU = mybir.AluOpType
ACTF = mybir.ActivationFunctionType

N_CORES = 8
N_ROWS = 2_000_000
IN_DIM = 64
HID = 128

# Device-side geometry (per core).
Q = 256                     # rows per score-block
CH_ROWS = 8192              # rows per DMA chunk ([128, 4096] f16 = 1 MiB)
N_CH = 31                   # chunks per core
R_CORE = N_CH * CH_ROWS     # 253952 rows per core
N_PAD = N_CORES * R_CORE    # 2031616 rows total (padded)
MM_PER_CH = 16              # mm1 matmuls (512 rows each) per chunk
N_MM = N_CH * MM_PER_CH     # 496
PH_MMS = 3                  # mm1 outputs per PSUM tile -> ACT N=1536
PH_COLS = 512 * PH_MMS
BLK_PER_HT = PH_COLS // Q   # 6 score blocks per ht tile
N_BLK = R_CORE // Q         # 992
N_ST = (N_BLK + 127) // 128  # 8 super-tiles (last partial: 96 blocks)
COLS = N_ST * Q             # 2048 e/out columns per partition


def _ap(handle_ap, offset, dims):
    """Custom access pattern on a DRAM tensor: dims = [[step, count], ...]."""
    return bass.AP(tensor=handle_ap.tensor, offset=offset, ap=list(dims))


# DVE-assisted tanh: every ASSIST_EVERY-th full ht tile is computed on the
# vector engine with a minimax R(5,4) rational instead of ACT (the kernel's
# bottleneck): tanh(z) ~ z*(t^2 + A2 t + A0)/(t^2 + B2 t + B0), t = z^2,
# fit on |z| <= 4.8; f16 end-to-end max err ~2.7e-3.  The assisted tile's
# mm2 blocks are deferred ASSIST_DEFER tiles so the in-order PE stream is
# not supposed to wait on the DVE chain.  Measured on HW: a net WASH --
# the Tile scheduler places the deferred blocks at the SIM's (optimistic)
# chain-end plus PE runs ~1.5us ahead of ACT, so each assist still stalls
# PE/ACT ~1.4us, cancelling the 1.42us tanh saving (tile_wait_until has no
# effect in this path; an ACT-copy gate made it worse: 2.5us stall/assist).
ASSIST_EVERY = 0            # 0 disables; 11 + DEFER 12 was the best tried
ASSIST_START = 8
ASSIST_STOP = 148
ASSIST_DEFER = 12
A2, A0 = -97.93266143005741, -1801.597436686564
B2, B0 = -680.3934051370769, -1811.5100012510918


def _is_assist(k):
    return (
        ASSIST_EVERY > 0
        and ASSIST_START <= k < ASSIST_STOP
        and (k - ASSIST_START) % ASSIST_EVERY == 0
    )


def build_nc(n_cores=N_CORES, use_coll=True):
    """Build the per-core Bass program (SPMD: same program, sliced inputs)."""
    from contextlib import ExitStack

    nc = bacc.Bacc(num_devices=n_cores)

    x_in = nc.declare_dram_parameter("x", [N_CH, 128, 4096], F16, isOutput=False)
    w1t_in = nc.declare_dram_parameter("w1t", [IN_DIM, HID], F16, isOutput=False)
    w2s_in = nc.declare_dram_parameter("w2s", [HID, 32 * 32], F16, isOutput=False)
    b1_in = nc.declare_dram_parameter("b1", [HID], F32, isOutput=False)
    b2_in = nc.declare_dram_parameter("b2", [1], F32, isOutput=False)
    oh_in = nc.declare_dram_parameter("oh", [2], F32, isOutput=False)
    pc2_in = nc.declare_dram_parameter("padc2", [2], F32, isOutput=False)
    out_t = nc.declare_dram_parameter("out", [128 * COLS], F32, isOutput=True)
    gs_t = nc.declare_dram_parameter("gsums", [2], F32, isOutput=True)

    cc_in = nc.dram_tensor("cc_in", [2], F32)
    cc_out = nc.dram_tensor("cc_out", [2], F32, addr_space="Shared")

    with ExitStack() as ctx:
        tc = ctx.enter_context(tile.TileContext(nc))
        singles = ctx.enter_context(tc.tile_pool(name="singles", bufs=1))
        xx_pool = ctx.enter_context(tc.tile_pool(name="xx", bufs=3))
        ht_pool = ctx.enter_context(
            tc.tile_pool(name="ht", bufs=(ASSIST_DEFER + 7) if ASSIST_EVERY else 6)
        )
        dv_pool = ctx.enter_context(tc.tile_pool(name="dv", bufs=1))
        gate_pool = ctx.enter_context(tc.tile_pool(name="gate", bufs=2))
        ph_pool = ctx.enter_context(tc.tile_pool(name="ph", bufs=2, space="PSUM"))
        ps_pool = ctx.enter_context(tc.tile_pool(name="ps", bufs=2, space="PSUM"))

        # ---- static setup ----------------------------------------------
        # First x chunk before anything else on the sync HWDGE queue; a
        # small leading slice so the first matmul can start earlier.
        xx_tiles = {}
        xx_tiles[0] = xx_pool.tile([128, 4096], F16, tag="xx", name="xx")
        w1t_sb = singles.tile([128, HID], F16)     # both halves hold W1T
        nc.sync.dma_start(
            out=w1t_sb[:],
            in_=_ap(w1t_in[:], 0, [[0, 2], [HID, IN_DIM], [1, HID]]),
        )
        # Fast-start slices: ph tile 0's mm1 reads (cols 0:1024), split
        # across both HWDGE queues.
        nc.sync.dma_start(
            out=xx_tiles[0][:, 0:512],
            in_=_ap(x_in[:], 0, [[4096, 128], [1, 512]]),
        )
        b1_sb = singles.tile([128, 1], F32)
        nc.sync.dma_start(out=b1_sb[:], in_=_ap(b1_in[:], 0, [[1, HID], [1, 1]]))
        b2_sb = singles.tile([128, 1], F32)
        nc.sync.dma_start(out=b2_sb[:], in_=_ap(b2_in[:], 0, [[0, 128], [1, 1]]))
        xx_tiles[1] = xx_pool.tile([128, 4096], F16, tag="xx", name="xx")
        nc.sync.dma_start(
            out=xx_tiles[1][:],
            in_=_ap(x_in[:], 128 * 4096, [[4096, 128], [1, 4096]]),
        )

        # Scalar HWDGE queue: rest of chunk 0, strips, small params.
        nc.scalar.dma_start(
            out=xx_tiles[0][:, 512:1024],
            in_=_ap(x_in[:], 512, [[4096, 128], [1, 512]]),
        )
        # 32 strip matrices [128, 32] fp16, strip c has W2 in column c.
        strips = singles.tile([128, 32, 32], F16)
        nc.scalar.dma_start(
            out=strips[:], in_=_ap(w2s_in[:], 0, [[32 * 32, HID], [1, 32 * 32]])
        )
        nc.scalar.dma_start(
            out=xx_tiles[0][:, 1024:4096],
            in_=_ap(x_in[:], 1024, [[4096, 128], [1, 3072]]),
        )
        oh1_sb = singles.tile([1, 2], F32)
        nc.scalar.dma_start(out=oh1_sb[:], in_=_ap(oh_in[:], 0, [[2, 1], [1, 2]]))
        pc2_sb = singles.tile([1, 2], F32)
        nc.scalar.dma_start(out=pc2_sb[:], in_=_ap(pc2_in[:], 0, [[2, 1], [1, 2]]))

        e_sb = singles.tile([128, COLS], F32)
        out_sb = singles.tile([128, COLS], F32)
        rr_tot = singles.tile([128, N_ST // 2 + 1], F32)
        rr1 = singles.tile([128, 1], F32)
        rr_red = singles.tile([128, 1], F32)
        ones_sb = singles.tile([128, 1], F32)
        ones_row = singles.tile([1, 128], F32)
        tiny = singles.tile([128, 1], F32)
        g2 = singles.tile([1, 2], F32)
        cg_sb = singles.tile([1, 2], F32)
        sg1 = singles.tile([1, 2], F32)
        s_g1 = singles.tile([1, 1], F32)
        inv1 = singles.tile([1, 1], F32)
        inv_g = singles.tile([128, 1], F32)

        nc.vector.memset(ones_sb[:], 1.0)
        nc.vector.memset(ones_row[:], 1.0)
        # rr_tot's last column is only written on 96 partitions.
        nc.vector.memset(rr_tot[:], 0.0)
        # Zero the unused corner of E (last super-tile has 96 blocks).
        nc.vector.memset(e_sb[96:128, (N_ST - 1) * Q : N_ST * Q], 0.0)
        # Dummy activation: pulls ACT_TABLE_LOAD off the critical path.
        nc.scalar.activation(
            out=tiny[:], in_=ones_sb[:], func=ACTF.Tanh, bias=0.0, scale=1.0
        )
        if use_coll:
            # Warmup AllReduce: pulls the CC-path cold start off the tail
            # and aligns the cores early.
            warm = singles.tile([1, 2], F32)
            cc_win = nc.dram_tensor("cc_win", [2], F32)
            cc_wout = nc.dram_tensor("cc_wout", [2], F32, addr_space="Shared")
            nc.vector.memset(warm[:], 0.0)
            nc.gpsimd.dma_start(out=cc_win[:], in_=warm[0:1, :])
            nc.gpsimd.collective_compute(
                "AllReduce",
                ALU.add,
                replica_groups=[list(range(n_cores))],
                ins=[cc_win[:]],
                outs=[cc_wout[:]],
            )

        # ---- phase 1: mm1 / tanh / mm2 / exp ---------------------------
        # Global mm1 index m: chunk ch=m//16, j=m%16 -> half=j%2, t=j//2.
        ph = None
        next_blk = 0
        MM2_DEFER = 2  # ht tiles between tanh and its mm2 emission

        # mm2 blocks of a DVE-assisted tile are emitted ASSIST_DEFER tiles
        # late (so the in-order PE stream never waits on the DVE tanh
        # chain).  PSUM accumulate flags: start on the FIRST-emitted block
        # of each 32-strip group (clears that quadrant region only --
        # region-scoped, verified on HW), stop on the last-emitted.
        N_GRP = N_BLK // 32  # 31 (exact)
        grp_cnt = [0] * N_GRP
        pair_tiles = {}
        pair_cnt = {}
        PAIR_LAST = (N_BLK - 1) // 256  # 3
        PAIR_SIZE = [min(N_BLK - 256 * P, 256) for P in range(PAIR_LAST + 1)]

        def emit_exp(P):
            sp = pair_tiles[P]
            nfull = min(PAIR_SIZE[P], 128)
            ntail = PAIR_SIZE[P] - 128
            if ntail == 128 or ntail <= 0:
                ncl = 2 * Q if ntail == 128 else Q
                nc.scalar.activation(
                    out=e_sb[0:nfull, P * 2 * Q : P * 2 * Q + ncl],
                    in_=sp[0:nfull, 0:ncl],
                    func=ACTF.Exp,
                    bias=b2_sb[0:nfull, :],
                    scale=1.0,
                    accum_out=rr_tot[0:nfull, P : P + 1],
                )
            else:
                nc.scalar.activation(
                    out=e_sb[:, P * 2 * Q : P * 2 * Q + Q],
                    in_=sp[:, 0:Q],
                    func=ACTF.Exp,
                    bias=b2_sb[:],
                    scale=1.0,
                    accum_out=rr_tot[:, P : P + 1],
                )
                nc.scalar.activation(
                    out=e_sb[0:ntail, P * 2 * Q + Q : (P + 1) * 2 * Q],
                    in_=sp[0:ntail, Q : 2 * Q],
                    func=ACTF.Exp,
                    bias=b2_sb[0:ntail, :],
                    scale=1.0,
                    accum_out=rr_tot[0:ntail, PAIR_LAST + 1 : PAIR_LAST + 2],
                )
            del pair_tiles[P]

        def emit_block(b, lhs=None):
            c = b % 32
            g = (b % 128) // 32
            gid = b // 32
            P = b // 256
            if P not in pair_tiles:
                pair_tiles[P] = ps_pool.tile(
                    [128, 2 * Q], F32, tag="score", name="score"
                )
            cb = ((b % 256) // 128) * Q
            k = b // BLK_PER_HT
            hoff = Q * (b % BLK_PER_HT)
            nc.tensor.matmul(
                pair_tiles[P][32 * g : 32 * g + 32, cb : cb + Q],
                strips[:, c, :] if lhs is None else lhs,
                ht_tiles[k][:, hoff : hoff + Q],
                start=(grp_cnt[gid] == 0),
                stop=(grp_cnt[gid] == 31),
                skip_group_check=True,
                tile_position=(0, 32 * g),
            )
            grp_cnt[gid] += 1
            pair_cnt[P] = pair_cnt.get(P, 0) + 1
            if pair_cnt[P] == PAIR_SIZE[P]:
                emit_exp(P)

        deferred = []  # (flush_tile, block) in block order
        gate_tiles = {}  # assist tile k -> (gate tile, c0)

        def flush_blocks(j):
            # The scheduler pops ready deferred blocks at any PE-idle
            # moment (its DVE chain estimate is optimistic, and
            # tile_wait_until has no effect in this path), so gate them on
            # an ACT-produced copy of their strip weights: the ACT queue
            # is saturated and priority-dense, so the copy executes at its
            # emission slot (after tanh j-1) -- a true cross-engine clock.
            while deferred and deferred[0][0] <= j:
                f, b = deferred.pop(0)
                k = b // BLK_PER_HT
                if k not in gate_tiles:
                    c0 = (BLK_PER_HT * k) % 32
                    gt = gate_pool.tile([128, 6, 32], F16, tag="gate", name="gt")
                    n1 = min(6, 32 - c0)
                    nc.scalar.activation(
                        out=gt[:, 0:n1, :], in_=strips[:, c0 : c0 + n1, :],
                        func=ACTF.Copy, bias=0.0, scale=1.0,
                    )
                    if n1 < 6:
                        nc.scalar.activation(
                            out=gt[:, n1:6, :], in_=strips[:, 0 : 6 - n1, :],
                            func=ACTF.Copy, bias=0.0, scale=1.0,
                        )
                    gate_tiles[k] = (gt, c0)
                gt, c0 = gate_tiles[k]
                idx = (b % 32 - c0) % 32
                emit_block(b, lhs=gt[:, idx, :])

        def emit_mm2_upto(bmax):
            nonlocal next_blk
            while next_blk < bmax:
                k = next_blk // BLK_PER_HT
                if _is_assist(k):
                    # spread the deferred flush 2 blocks/tile (PE slack is
                    # only ~0.25us/tile -- a 6-block lump starves ACT)
                    off = (next_blk - BLK_PER_HT * k) // 2
                    deferred.append((k + ASSIST_DEFER + off, next_blk))
                else:
                    emit_block(next_blk)
                next_blk += 1

        ht_tiles = {}
        for m in range(N_MM):
            ch, j = divmod(m, MM_PER_CH)
            if j == 4 and ch + 2 < N_CH and (ch + 2) not in xx_tiles:
                ch2 = ch + 2
                q = nc.sync if ch2 % 2 == 0 else nc.scalar
                xx_tiles[ch2] = xx_pool.tile([128, 4096], F16, tag="xx", name="xx")
                q.dma_start(
                    out=xx_tiles[ch2][:],
                    in_=_ap(x_in[:], ch2 * 128 * 4096, [[4096, 128], [1, 4096]]),
                )
            if j == 0 and ch not in xx_tiles:
                q = nc.sync if ch % 2 == 0 else nc.scalar
                xx_tiles[ch] = xx_pool.tile([128, 4096], F16, tag="xx", name="xx")
                q.dma_start(
                    out=xx_tiles[ch][:],
                    in_=_ap(x_in[:], ch * 128 * 4096, [[4096, 128], [1, 4096]]),
                )
            half, t = j % 2, j // 2
            slot = m % PH_MMS
            if slot == 0:
                ph = ph_pool.tile([128, PH_COLS], F32, tag="ph")
            nc.tensor.matmul(
                ph[:, 512 * slot : 512 * (slot + 1)],
                w1t_sb[64 * half : 64 * half + 64, :],
                xx_tiles[ch][64 * half : 64 * half + 64, 512 * t : 512 * (t + 1)],
                start=True,
                stop=True,
            )
            if slot == PH_MMS - 1 or m == N_MM - 1:
                k = m // PH_MMS
                ncols = 512 * (slot + 1)
                ht_tiles[k] = ht_pool.tile([128, PH_COLS], F16, tag="ht", name="ht")
                hk = ht_tiles[k]
                if _is_assist(k) and ncols == PH_COLS:
                    # DVE rational tanh: z = ph + b1; t = z^2; t4 = t^2;
                    # ht = z*(t4 + A2 t + A0) / (t4 + B2 t + B0)
                    zs = dv_pool.tile([128, PH_COLS], F16, tag="dv_zs", name="zs")
                    tq = dv_pool.tile([128, PH_COLS], F16, tag="dv_t", name="tq")
                    t4 = dv_pool.tile([128, PH_COLS], F16, tag="dv_t4", name="t4")
                    nm0 = dv_pool.tile([128, PH_COLS], F16, tag="dv_n0", name="nm0")
                    nm1 = dv_pool.tile([128, PH_COLS], F16, tag="dv_n1", name="nm1")
                    d0 = dv_pool.tile([128, PH_COLS], F16, tag="dv_d0", name="d0")
                    dd = dv_pool.tile([128, PH_COLS], F32, tag="dv_dd", name="dd")
                    rcp = dv_pool.tile([128, PH_COLS], F32, tag="dv_r", name="rcp")
                    # TS/TT only (no STT): the scheduler's DVE cost model
                    # prices STT at 2x but HW runs it at 1x -- the sim/HW
                    # divergence made the scheduler place deferred mm2s too
                    # early in the PE stream.  TS f16 runs 4x, TT f16 2x on
                    # both, so sim chain length ~= HW chain length (~12.6us).
                    aux = dv_pool.tile([128, PH_COLS], F16, tag="dv_x", name="aux")
                    with nc.allow_low_precision(reason="f16 rational tanh"):
                        nc.vector.tensor_scalar(
                            out=zs[:], in0=ph[:], scalar1=b1_sb[:],
                            scalar2=None, op0=ALU.add,
                        )
                        nc.vector.tensor_mul(tq[:], zs[:], zs[:])
                        nc.vector.tensor_mul(t4[:], tq[:], tq[:])
                        nc.vector.tensor_scalar(
                            out=aux[:], in0=tq[:], scalar1=float(A2),
                            scalar2=None, op0=ALU.mult,
                        )
                        nc.vector.tensor_add(nm0[:], aux[:], t4[:])
                        nc.vector.tensor_scalar(
                            out=aux[:], in0=nm0[:], scalar1=float(A0),
                            scalar2=None, op0=ALU.add,
                        )
                        nc.vector.tensor_mul(nm1[:], aux[:], zs[:])
                        nc.vector.tensor_scalar(
                            out=aux[:], in0=tq[:], scalar1=float(B2),
                            scalar2=None, op0=ALU.mult,
                        )
                        nc.vector.tensor_add(d0[:], aux[:], t4[:])
                        nc.vector.tensor_scalar(
                            out=dd[:], in0=d0[:], scalar1=float(B0),
                            scalar2=None, op0=ALU.add,
                        )
                        nc.vector.reciprocal_approx_fast(out=rcp[:], in_=dd[:])
                        nc.vector.tensor_mul(hk[:], nm1[:], rcp[:])
                else:
                    nc.scalar.activation(
                        out=hk[:, 0:ncols],
                        in_=ph[:, 0:ncols],
                        func=ACTF.Tanh,
                        bias=b1_sb[:],
                        scale=1.0,
                    )
                flush_blocks(k)
                if k >= MM2_DEFER:
                    emit_mm2_upto(((k - MM2_DEFER + 1) * PH_COLS) // Q)
        emit_mm2_upto(N_BLK)
        flush_blocks(10**9)

        # ---- core partial sums + allreduce ------------------------------
        nc.vector.reduce_sum(rr1[:], rr_tot[:], axis=mybir.AxisListType.X)
        ps_rr = ps_pool.tile([128, 2], F32, tag="score")
        nc.tensor.matmul(ps_rr[0:1, 0:1], ones_sb[:], rr1[:], start=True, stop=True)
        nc.scalar.activation(
            out=rr_red[0:1, :], in_=ps_rr[0:1, 0:1], func=ACTF.Copy, bias=0.0,
            scale=1.0,
        )
        # g2 = onehot * S_core - padc2   (padc2 = onehot * n_pad * e_pad)
        nc.vector.scalar_tensor_tensor(
            out=g2[0:1, :],
            in0=oh1_sb[0:1, :],
            scalar=rr_red[0:1, 0:1],
            in1=pc2_sb[0:1, :],
            op0=ALU.mult,
            op1=ALU.subtract,
        )

        if use_coll:
            nc.gpsimd.dma_start(out=cc_in[:], in_=g2[0:1, :])
            nc.gpsimd.collective_compute(
                "AllReduce",
                ALU.add,
                replica_groups=[list(range(n_cores))],
                ins=[cc_in[:]],
                outs=[cc_out[:]],
            )
            nc.scalar.dma_start(out=gs_t[:], in_=cc_out[:])
            nc.sync.dma_start(
                out=cg_sb[:], in_=_ap(cc_out[:], 0, [[2, 1], [1, 2]])
            )
        else:
            nc.sync.dma_start(out=gs_t[:], in_=g2[0:1, :])
            nc.sync.dma_start(
                out=cg_sb[:], in_=_ap(gs_t[:], 0, [[2, 1], [1, 2]])
            )

        # ---- normalize + store -----------------------------------------
        # inv_g = 1 / (my group's global sum), broadcast to 128 partitions
        # via a K=1 ones matmul; quarters on alternating HWDGE queues so
        # stores overlap the next quarter's normalize.
        nc.vector.tensor_mul(sg1[:], cg_sb[:], oh1_sb[:])
        nc.vector.reduce_sum(s_g1[:], sg1[:], axis=mybir.AxisListType.X)
        nc.vector.reciprocal(out=inv1[:], in_=s_g1[:])
        ps_b = ps_pool.tile([128, 2], F32, tag="score")
        nc.tensor.matmul(
            ps_b[:, 0:1], ones_row[0:1, :], inv1[0:1, :], start=True, stop=True
        )
        nc.scalar.activation(
            out=inv_g[:], in_=ps_b[:, 0:1], func=ACTF.Copy, bias=0.0, scale=1.0
        )
        NQ = 4
        H = COLS // NQ
        for i in range(NQ):
            eng = nc.sync if i % 2 == 0 else nc.scalar
            sl = slice(i * H, (i + 1) * H)
            nc.vector.tensor_scalar(
                out=out_sb[:, sl],
                in0=e_sb[:, sl],
                scalar1=inv_g[:],
                scalar2=None,
                op0=ALU.mult,
            )
            eng.dma_start(
                out=_ap(out_t[:], i * H, [[COLS, 128], [1, H]]),
                in_=out_sb[:, sl],
            )

    nc.compile()
    return nc


_NC_CACHE = {}


def _get_nc(key=0):
    if key not in _NC_CACHE:
        _NC_CACHE[key] = build_nc()
    return _NC_CACHE[key]


# Host-side index map (identical for every core): for actual row a in
# [0, R_CORE): position in the device (p, col) layout.
_IDX_CACHE = {}


def _layout_index():
    """Return (p, col) arrays mapping core-row a -> device layout slot."""
    if "idx" not in _IDX_CACHE:
        a = np.arange(R_CORE, dtype=np.int64)
        ch, o = a // CH_ROWS, a % CH_ROWS
        half, w = o // 4096, o % 4096
        t, jr = w // 512, w % 512
        m = MM_PER_CH * ch + 2 * t + half
        q = 512 * m + jr
        b, r = q // Q, q % Q
        p = b % 128
        col = (b // 128) * Q + r
        _IDX_CACHE["idx"] = (p, col)
    return _IDX_CACHE["idx"]


def prep_inputs(x, T, W1, b1, W2, b2, n_cores=N_CORES):
    """Host-side shard/layout prep -> (per-core input maps, perm).

    Rows are stably partitioned by group: cores 0..k0-1 get group-0 rows,
    cores k0..7 group-1, with zero-padding at the two group tails so every
    core is group-pure.
    """
    n_rows = x.shape[0]
    assert n_rows == N_ROWS

    T = np.asarray(T).astype(np.int64).reshape(-1)
    perm = np.argsort(T, kind="stable")
    g0 = int((T == 0).sum())
    g1 = n_rows - g0
    k0 = int(np.ceil(g0 / R_CORE))
    assert k0 * R_CORE >= g0 and (n_cores - k0) * R_CORE >= g1, (
        "group sizes too imbalanced for group-pure sharding"
    )

    x = np.asarray(x, dtype=np.float32)
    xg = x[perm].astype(np.float16)
    xh = np.zeros((N_PAD, IN_DIM), dtype=np.float16)
    xh[:g0] = xg[:g0]
    xh[k0 * R_CORE : k0 * R_CORE + g1] = xg[g0:]
    del xg
    # [n_chunk_tot, 2, 4096, 64] -> [n_chunk_tot, 2, 64, 4096] -> [., 128, 4096]
    n_ch_tot = N_PAD // CH_ROWS
    xd = np.ascontiguousarray(
        xh.reshape(n_ch_tot, 2, 4096, IN_DIM)
        .transpose(0, 1, 3, 2)
        .reshape(n_ch_tot, 128, 4096)
    )

    w1t = np.ascontiguousarray(np.asarray(W1, np.float32).T).astype(np.float16)
    w2s = np.zeros((HID, 32, 32), dtype=np.float16)
    w2v = np.asarray(W2, np.float32).reshape(HID).astype(np.float16)
    for c in range(32):
        w2s[:, c, c] = w2v
    w2s = w2s.reshape(HID, 32 * 32)
    b1h = np.asarray(b1, np.float32).reshape(HID).copy()
    b2h = np.asarray(b2, np.float32).reshape(1).copy()

    # Host estimate of the device's E value on padded rows (x = 0).
    h_pad = np.tanh(b1h)
    e_pad = float(np.exp(b2h[0] + np.asarray(W2, np.float32).reshape(HID) @ h_pad))

    in_maps = []
    for cid in range(n_cores):
        grp = 0 if cid < k0 else 1
        if cid < k0:
            n_real = min(max(g0 - cid * R_CORE, 0), R_CORE)
        else:
            n_real = min(max(g1 - (cid - k0) * R_CORE, 0), R_CORE)
        n_pad_c = R_CORE - n_real
        oh = np.zeros(2, dtype=np.float32)
        oh[grp] = 1.0
        padc2 = oh * (n_pad_c * e_pad)
        in_maps.append(
            {
                "x": xd[cid * N_CH : (cid + 1) * N_CH],
                "w1t": w1t,
                "w2s": w2s,
                "b1": b1h,
                "b2": b2h,
                "oh": oh,
                "padc2": padc2,
            }
        )
    return in_maps, perm, g0, k0


def run(x, T, W1, b1, W2, b2, trace=False):
    in_maps, perm, g0, k0 = prep_inputs(x, T, W1, b1, W2, b2)
    nc = _get_nc()
    res = run_bass_kernel_spmd(nc, in_maps, list(range(N_CORES)), trace=trace)
    p_idx, c_idx = _layout_index()
    n_rows = x.shape[0]
    g1 = n_rows - g0
    # Collect per-core real rows (device layout -> permuted row order).
    vals = np.empty(N_PAD, dtype=np.float32)
    for cid in range(N_CORES):
        lay = res.results[cid]["out"].reshape(128, COLS)
        vals[cid * R_CORE : (cid + 1) * R_CORE] = lay[p_idx, c_idx]
    out = np.empty(n_rows, dtype=np.float32)
    out[perm[:g0]] = vals[:g0]
    out[perm[g0:]] = vals[k0 * R_CORE : k0 * R_CORE + g1]
    return out, res


def kernel(x, T, W1, b1, W2, b2):
    out, _ = run(x, T, W1, b1, W2, b2)
    return out
